# revision 1
# baseline (speedup 1.0000x reference)
"""CQAttention (QANet context-query attention) Bass kernel for 8 Trainium2 cores.

Math (per batch, masks all-ones, eval mode):
  Ct = C.T [Lc,D], Qt = Q.T [Lq,D]
  S  = Ct@w4C + (Qt@w4Q).T + (Ct*w4mlu)@Qt.T + bias          [Lc,Lq]
  S1 = softmax_q(S), S2 = softmax_c(S)
  A  = S1@Qt ; Bt = S1@(S2.T@Ct)
  out = concat([Ct, A, Ct*A, Ct*Bt], -1).T                    [4D, Lc]

Key reductions used here:
  - (S1@S2.T)@Ct re-associated as S1@(S2.T@Ct)  (6x fewer flops)
  - softmax terms constant along the reduced axis cancel, so:
      S1 = E1/r,  E1^T[q,c] = exp(sum_d Q[d,q]*Caug[d,c]),  Caug = C*w4mlu + w4Q
      S2 = E2/s,  E2[c,q]   = exp(sum_d C[d,c]*Qaug[d,q]),  Qaug = Q*w4mlu + w4C
    (bias and the remaining rank-1 terms cancel exactly in every output)
  - row-sums r / col-sums s replicated across partitions via ones-matmul
  - outputs stay in [d, c] layout end-to-end:
      out1 = MA*(1/r), out2 = out1*C, out3 = (MB*(1/r))*C
      MA = Qt.T @ E1^T, MB = T.T @ E1^T, T = transpose((Ct.T @ E2) * (1/s))

Schedule (the perf-critical part):
  - ALL input DMAs (4x C, 4x Q) are issued up front on the SP queue
    (first C load quarter-split so E2 starts early), then the 4 C
    passthrough planes stored from SBUF, then output stores in
    completion order.  SP is a pure-DMA engine here, so nothing
    compute-dependent ever blocks a load behind it: input DMA streams
    ahead of compute and the store stream drains behind it (~20.5 MB
    of HBM traffic per iteration).
  - C/Q DRAM params are declared float32r (same bits as fp32) so the
    DMA itself is the fp32r "producer" the BIR verifier requires;
    every PE operand then runs at 1 cycle/row (1.5 for transposes)
    with no rounding copies.
  - per-batch software pipelining: Ct and Caug for batch b+1 are
    computed at the end of batch b; within a batch the c-half-g0
    r/MA/store chain runs before the full-c s/MT reductions.
  - elementwise spread: Act = exps + Caug + Ct-half copy, DVE =
    reciprocals/psum drains + Ct-half copy, GpSimd = SBUF-only muls
    (GPSIMD cannot read PSUM on hardware).
"""

import numpy as np

import concourse.bass as bass
import concourse.bacc as bacc
import concourse.tile as tile
from concourse import mybir
from contextlib import ExitStack

B, D, LC, LQ = 32, 128, 2048, 256
NCORES = 8
BPC = B // NCORES  # batches per core

F32 = mybir.dt.float32
F32R = mybir.dt.float32r
AF = mybir.ActivationFunctionType
ALU = mybir.AluOpType

IO_BUFS = 4       # all batches' inputs prefetched
OUT_BUFS = 2
BIG_BUFS = 3
SMALL_BUFS = 2
WORK_BUFS = 1


def build_nc(reps=1, hw_loop=False):
    nc = bacc.Bacc("TRN2", target_bir_lowering=False)
    # C/Q declared float32r (same 32-bit layout as float32): the DMA loads
    # then land fp32r tiles directly, so PE can consume them with no
    # rounding copies; elementwise consumers use a bitcast-to-f32 view.
    C_in = nc.declare_dram_parameter("C", [BPC, D, LC], F32R, isOutput=False)
    Q_in = nc.declare_dram_parameter("Q", [BPC, D, LQ], F32R, isOutput=False)
    # packed [w4mlu | w4C | w4Q] so the prologue is one tiny DMA
    w4_in = nc.declare_dram_parameter("w4", [D, 3], F32, isOutput=False)
    out_ext = nc.declare_dram_parameter("out", [BPC, 4 * D, LC], F32, isOutput=True)

    with ExitStack() as ctx:
        tc = ctx.enter_context(tile.TileContext(nc))
        singles = ctx.enter_context(tc.tile_pool(name="singles", bufs=1))
        io = ctx.enter_context(tc.tile_pool(name="io", bufs=IO_BUFS))
        outp = ctx.enter_context(tc.tile_pool(name="outp", bufs=OUT_BUFS))
        work = ctx.enter_context(tc.tile_pool(name="work", bufs=WORK_BUFS))
        psum = ctx.enter_context(tc.tile_pool(name="psum", bufs=1, space="PSUM"))

        ident = singles.tile([128, 128], F32)
        nc.gpsimd.memset(ident, 0.0)
        nc.gpsimd.affine_select(
            out=ident, in_=ident, compare_op=ALU.not_equal, fill=1.0,
            base=0, pattern=[[-1, 128]], channel_multiplier=1)
        identr = singles.tile([128, 128], F32R)
        nc.vector.tensor_copy(out=identr, in_=ident)
        ones_f = singles.tile([128, 128], F32)
        nc.vector.memset(ones_f, 1.0)
        ones = singles.tile([128, 128], F32R)
        nc.vector.tensor_copy(out=ones, in_=ones_f)
        w4_sb = singles.tile([128, 3], F32)
        nc.sync.dma_start(out=w4_sb, in_=w4_in[:])
        w4mlu_sb = w4_sb[:, 0:1]
        w4C_sb = w4_sb[:, 1:2]
        w4Q_sb = w4_sb[:, 2:3]

        from contextlib import nullcontext
        loop_cm = (tc.For_i(0, reps, 1,
                            hint_engines=(mybir.EngineType.PE,
                                          mybir.EngineType.DVE,
                                          mybir.EngineType.Activation,
                                          mybir.EngineType.SP,
                                          mybir.EngineType.Pool))
                   if hw_loop else nullcontext(0))
        with loop_cm:
         for rep in range(1 if hw_loop else reps):
          # ---- prologue: all input loads, then DRAM->DRAM passthroughs ----
          Qsbs, Csbs = [], []
          for b in range(BPC):
              q = io.tile([128, LQ], F32R, tag="Qsb")
              nc.sync.dma_start(out=q, in_=Q_in[b])
              Qsbs.append(q)
              c = io.tile([128, LC], F32R, tag="Csb")
              if b == 0:
                  # quarter-split the critical first C load so each E2
                  # group can start as soon as its c-tiles land
                  for qtr in range(4):
                      nc.sync.dma_start(
                          out=c[:, 512 * qtr:512 * (qtr + 1)],
                          in_=C_in[b][:, 512 * qtr:512 * (qtr + 1)])
              else:
                  nc.sync.dma_start(out=c, in_=C_in[b])
              Csbs.append(c)
          # pass plane 0 fills the load->store transition; planes 1-3 are
          # issued at the batch seams (end of each batch's store section)
          # where the store stream would otherwise idle awaiting compute
          nc.sync.dma_start(out=out_ext[0, 0:128, :],
                            in_=Csbs[0][:].bitcast(F32))

          def make_ct(b):
              # Ct = C.T via 16 PE transposes; psum drained by DVE + Act
              # (GPSIMD cannot read PSUM on hardware).
              Cr = Csbs[b]
              Ct = work.tile([128, LC], F32R, tag="Ct", bufs=2)
              for g in range(2):
                  ps_ct = psum.tile([128, 1024], F32, tag="big", bufs=BIG_BUFS)
                  for j in range(8):
                      cj = g * 8 + j
                      nc.tensor.transpose(
                          ps_ct[:, 128 * j:128 * (j + 1)].bitcast(F32R),
                          Cr[:, 128 * cj:128 * (cj + 1)], identr)
                  if g == 0:
                      nc.vector.tensor_copy(
                          out=Ct[:, 1024 * g:1024 * (g + 1)], in_=ps_ct)
                  else:
                      nc.scalar.copy(
                          out=Ct[:, 1024 * g:1024 * (g + 1)], in_=ps_ct)
              return Ct

          def make_caug(b, engine):
              Caug = work.tile([128, LC], F32R, tag="Caug", bufs=2)
              if engine == "act":
                  nc.scalar.activation(
                      out=Caug, in_=Csbs[b][:].bitcast(F32), func=AF.Identity,
                      scale=w4mlu_sb, bias=w4Q_sb)
              else:
                  nc.vector.tensor_scalar(
                      out=Caug, in0=Csbs[b][:].bitcast(F32), scalar1=w4mlu_sb,
                      scalar2=w4Q_sb, op0=ALU.mult, op1=ALU.add)
              return Caug

          # batch-0 Caug on DVE so Act's exp stream starts unblocked
          Caug_next = make_caug(0, "dve")
          Ct_next = None  # batch 0 computes its own Ct after E1

          for b in range(BPC):
              Cr, Qr = Csbs[b], Qsbs[b]
              Caug, Ct = Caug_next, Ct_next

              # Qaug on DVE (small, unblocks E2)
              Qaug = work.tile([128, LQ], F32R, tag="Qaug")
              nc.vector.tensor_scalar(
                  out=Qaug, in0=Qr[:].bitcast(F32), scalar1=w4mlu_sb,
                  scalar2=w4C_sb, op0=ALU.mult, op1=ALU.add)

              if b == 0:
                  # PE is otherwise idle waiting for C0: do Qt now
                  Qt0 = work.tile([128, LQ], F32R, tag="Qt")
                  ps_qt0 = psum.tile([128, 512], F32, tag="small",
                                     bufs=SMALL_BUFS)
                  for j in range(2):
                      nc.tensor.transpose(
                          ps_qt0[:, 128 * j:128 * (j + 1)].bitcast(F32R),
                          Qr[:, 128 * j:128 * (j + 1)], identr)
                  nc.vector.tensor_copy(out=Qt0, in_=ps_qt0[:, 0:256])

              # ---- E2[c,q] = exp(C.T @ Qaug), one tile per 4-c-tile group ----
              E2t = []
              for g in range(4):
                  ps = psum.tile([128, 1024], F32, tag="big", bufs=BIG_BUFS)
                  for j in range(4):
                      ctile = g * 4 + j
                      nc.tensor.matmul(
                          ps[:, 256 * j:256 * (j + 1)],
                          Cr[:, 128 * ctile:128 * (ctile + 1)], Qaug,
                          start=True, stop=True)
                  e = work.tile([128, 4 * LQ], F32R, tag=f"E2g{g}")
                  nc.scalar.activation(out=e, in_=ps, func=AF.Exp)
                  E2t.append(e)

              # ---- E1^T[q,c] = exp(Q.T @ Caug), tiles keyed (qt, c-half) ----
              # After both q-tiles of a half land, gpsimd presums them so the
              # r ones-matmul needs a single pass (half the PE cycles).
              E1t = {}
              for g in range(2):
                  for qt in range(2):
                      ps = psum.tile([128, 1024], F32, tag="big", bufs=BIG_BUFS)
                      for cc in range(2):
                          c0 = 1024 * g + 512 * cc
                          nc.tensor.matmul(
                              ps[:, 512 * cc:512 * (cc + 1)],
                              Qr[:, 128 * qt:128 * (qt + 1)],
                              Caug[:, c0:c0 + 512],
                              start=True, stop=True)
                      e = work.tile([128, 1024], F32R, tag=f"E1_{qt}{g}")
                      nc.scalar.activation(out=e, in_=ps, func=AF.Exp)
                      E1t[(qt, g)] = e

              # Qt transposes sit after E1 so E2/E1 launch ASAP at batch start
              if b == 0:
                  Qt = Qt0
              else:
                  Qt = work.tile([128, LQ], F32R, tag="Qt")
                  ps_qt = psum.tile([128, 512], F32, tag="small",
                                    bufs=SMALL_BUFS)
                  for j in range(2):
                      nc.tensor.transpose(
                          ps_qt[:, 128 * j:128 * (j + 1)].bitcast(F32R),
                          Qr[:, 128 * j:128 * (j + 1)], identr)
                  nc.vector.tensor_copy(out=Qt, in_=ps_qt[:, 0:256])

              if Ct is None:
                  Ct = make_ct(b)

              outs = outp.tile([128, 3, LC], F32, tag="outs")
              rbi = work.tile([128, LC], F32, tag="rbi")
              last = b == BPC - 1

              def do_r(g):
                  # replicated row-sums of E1 over q -> rbi = 1/r (c-half g)
                  ps = psum.tile([128, 1024], F32, tag="big", bufs=BIG_BUFS)
                  for cc in range(2):
                      for qt in range(2):
                          nc.tensor.matmul(
                              ps[:, 512 * cc:512 * (cc + 1)],
                              ones, E1t[(qt, g)][:, 512 * cc:512 * (cc + 1)],
                              start=(qt == 0), stop=(qt == 1))
                  nc.vector.reciprocal_approx_fast(
                      out=rbi[:, 1024 * g:1024 * (g + 1)], in_=ps)

              def do_ma(g):
                  # MA half -> out1 (DVE), out2 (gpsimd), store the half
                  sl = slice(1024 * g, 1024 * (g + 1))
                  ps = psum.tile([128, 1024], F32, tag="big", bufs=BIG_BUFS)
                  for cc in range(2):
                      for qt in range(2):
                          nc.tensor.matmul(
                              ps[:, 512 * cc:512 * (cc + 1)],
                              Qt[:, 128 * qt:128 * (qt + 1)],
                              E1t[(qt, g)][:, 512 * cc:512 * (cc + 1)],
                              start=(qt == 0), stop=(qt == 1))
                  nc.vector.tensor_mul(out=outs[:, 0, sl], in0=ps, in1=rbi[:, sl])
                  nc.sync.dma_start(
                      out=out_ext[b, 128:256, sl], in_=outs[:, 0, sl])
                  nc.gpsimd.tensor_mul(
                      out=outs[:, 1, sl], in0=outs[:, 0, sl],
                      in1=Cr[:, sl].bitcast(F32))
                  nc.sync.dma_start(
                      out=out_ext[b, 256:384, sl], in_=outs[:, 1, sl])

              def do_mb(g):
                  sl = slice(1024 * g, 1024 * (g + 1))
                  ps2 = psum.tile([128, 1024], F32, tag="big", bufs=BIG_BUFS)
                  for cc in range(2):
                      for qt in range(2):
                          nc.tensor.matmul(
                              ps2[:, 512 * cc:512 * (cc + 1)],
                              T_sb[:, 128 * qt:128 * (qt + 1)],
                              E1t[(qt, g)][:, 512 * cc:512 * (cc + 1)],
                              start=(qt == 0), stop=(qt == 1))
                  MBr = work.tile([128, 1024], F32, tag="MBr", bufs=2)
                  nc.vector.tensor_mul(out=MBr, in0=ps2, in1=rbi[:, sl])
                  nc.gpsimd.tensor_mul(
                      out=outs[:, 2, sl], in0=MBr,
                      in1=Cr[:, sl].bitcast(F32))
                  if last:
                      nc.sync.dma_start(
                          out=out_ext[b, 384:512, sl], in_=outs[:, 2, sl])

              # ---- c-half pipeline: g0's r/MA (and its stores) run before
              # the full-c reductions (s/MT) so output DMA starts early ----
              do_r(0)
              do_ma(0)

              # ---- s (col-sums of E2) and MT^T = Ct.T @ E2, interleaved by
              # group so PE consumption tracks the Act exp stream ----
              sinv = work.tile([128, LQ], F32, tag="sinv")
              MTs = work.tile([128, LQ], F32R, tag="MTs")
              ps_s = psum.tile([128, 512], F32, tag="small", bufs=SMALL_BUFS)
              ps_mt = psum.tile([128, 512], F32, tag="small", bufs=SMALL_BUFS)
              for g in range(4):
                  for j in range(4):
                      blk = 4 * g + j
                      nc.tensor.matmul(
                          ps_s[:, 0:256], ones, E2t[g][:, 256 * j:256 * (j + 1)],
                          start=(blk == 0), stop=(blk == 15))
                  for j in range(4):
                      blk = 4 * g + j
                      nc.tensor.matmul(
                          ps_mt[:, 0:256],
                          Ct[:, 128 * blk:128 * (blk + 1)],
                          E2t[g][:, 256 * j:256 * (j + 1)],
                          start=(blk == 0), stop=(blk == 15))
              do_r(1)
              do_ma(1)

              nc.vector.reciprocal_approx_fast(out=sinv, in_=ps_s[:, 0:256])
              nc.vector.tensor_mul(out=MTs, in0=ps_mt[:, 0:256], in1=sinv)

              # ---- T = transpose(MTs): [q, d] for MB's stationary operand ----
              T_sb = work.tile([128, LQ], F32R, tag="T_sb")
              ps_t = psum.tile([128, 512], F32, tag="small", bufs=SMALL_BUFS)
              for j in range(2):
                  nc.tensor.transpose(
                      ps_t[:, 128 * j:128 * (j + 1)].bitcast(F32R),
                      MTs[:, 128 * j:128 * (j + 1)], identr)
              nc.scalar.copy(out=T_sb, in_=ps_t[:, 0:256])

              do_mb(0)
              do_mb(1)

              # software-pipelined Ct/Caug for the next batch: PE fills the
              # gap while DVE drains the MB psums; Act appends Caug after
              # its exp stream so exps never queue behind it
              if b + 1 < BPC:
                  Ct_next = make_ct(b + 1)
                  Caug_next = make_caug(b + 1, "act")

              if not last:
                  nc.sync.dma_start(
                      out=out_ext[b, 384:512, :], in_=outs[:, 2, :])
                  nc.sync.dma_start(
                      out=out_ext[b + 1, 0:128, :],
                      in_=Csbs[b + 1][:].bitcast(F32))

    nc.compile()
    return nc


_NC = {}


def _get_nc(reps=1, hw_loop=False):
    key = (reps, hw_loop)
    if key not in _NC:
        _NC[key] = build_nc(reps, hw_loop)
    return _NC[key]


def make_in_maps(C, Q, w4C, w4Q, w4mlu):
    C = np.ascontiguousarray(np.asarray(C), dtype=np.float32)
    Q = np.ascontiguousarray(np.asarray(Q), dtype=np.float32)
    w4C = np.asarray(w4C, dtype=np.float32).reshape(D, 1)
    w4Q = np.asarray(w4Q, dtype=np.float32).reshape(D, 1)
    w4mlu = np.asarray(w4mlu, dtype=np.float32).reshape(D, 1)
    w4 = np.ascontiguousarray(np.concatenate([w4mlu, w4C, w4Q], axis=1))
    in_maps = []
    for i in range(NCORES):
        sl = slice(i * BPC, (i + 1) * BPC)
        in_maps.append({
            "C": np.ascontiguousarray(C[sl]),
            "Q": np.ascontiguousarray(Q[sl]),
            "w4": w4,
        })
    return in_maps


def run(C, Q, w4C, w4Q, w4mlu, trace=False, tmpdir=None):
    from concourse.bass_utils import run_bass_kernel_spmd
    nc = _get_nc()
    in_maps = make_in_maps(C, Q, w4C, w4Q, w4mlu)
    res = run_bass_kernel_spmd(
        nc, in_maps, list(range(NCORES)), trace=trace, tmpdir=tmpdir)
    out = np.concatenate(
        [res.results[i]["out"] for i in range(NCORES)], axis=0)
    return out, res


def kernel(C, Q, Cmask=None, Qmask=None, w4C=None, w4Q=None, w4mlu=None,
           bias=None, **_unused):
    # Cmask/Qmask are all-ones in this problem and bias cancels exactly in
    # every output (softmax shift invariance), so neither reaches the device.
    out, _ = run(C, Q, w4C, w4Q, w4mlu)
    return out



# revision 36
# speedup vs baseline: 2.6086x; 2.6086x over previous
"""CQAttention (QANet context-query attention) Bass kernel for 8 Trainium2 cores.

Math (per batch, masks all-ones, eval mode):
  Ct = C.T [Lc,D], Qt = Q.T [Lq,D]
  S  = Ct@w4C + (Qt@w4Q).T + (Ct*w4mlu)@Qt.T + bias          [Lc,Lq]
  S1 = softmax_q(S), S2 = softmax_c(S)
  A  = S1@Qt ; Bt = S1@(S2.T@Ct)
  out = concat([Ct, A, Ct*A, Ct*Bt], -1).T                    [4D, Lc]

Key reductions used here:
  - (S1@S2.T)@Ct re-associated as S1@(S2.T@Ct)  (6x fewer flops)
  - softmax terms constant along the reduced axis cancel, so:
      S1 = E1/r,  E1^T[q,c] = exp(sum_d Qmlu[d,q]*C[d,c] + sub1[q]),
                  Qmlu = Q*w4mlu,  sub1 = Q.T@w4Q (per-q Act exp bias)
      S2 = E2/s,  E2[c,q]   = exp(sum_d C[d,c]*Qaug[d,q]),  Qaug = Q*w4mlu + w4C
    (bias and the remaining rank-1 terms cancel exactly in every output;
    writing E1's rank-1 q-term as an exp bias removes the [128,Lc] Caug
    tile and its 2048-elem activation per batch entirely -- E1 reads the
    raw C tile, which also unblocks the first batch earlier)
  - row-sums r / col-sums s replicated across partitions via ones-matmul
  - outputs stay in [d, c] layout end-to-end:
      out1 = MA*(1/r), out2 = out1*C, out3 = (MB*(1/r))*C
      MA = Qt.T @ E1^T, MB = T.T @ E1^T, T = transpose((Ct.T @ E2) * (1/s))

Schedule (the perf-critical part):
  - ALL input DMAs (4x C, 4x Q) are issued up front on the SP queue
    (first C load quarter-split so E2 starts early), then the 4 C
    passthrough planes stored from SBUF, then output stores in
    completion order.  SP is a pure-DMA engine here, so nothing
    compute-dependent ever blocks a load behind it: input DMA streams
    ahead of compute and the store stream drains behind it (~20.5 MB
    of HBM traffic per iteration).
  - C/Q DRAM params are declared float32r (same bits as fp32) so the
    DMA itself is the fp32r "producer" the BIR verifier requires;
    every PE operand then runs at 1 cycle/row (1.5 for transposes)
    with no rounding copies.
  - per-batch software pipelining: Ct and Caug for batch b+1 are
    computed at the end of batch b; within a batch the c-half-g0
    r/MA/store chain runs before the full-c s/MT reductions.
  - elementwise spread: Act = exps + Caug + Ct-half copy, DVE =
    reciprocals/psum drains + Ct-half copy, GpSimd = SBUF-only muls
    (GPSIMD cannot read PSUM on hardware).
"""

import numpy as np

import concourse.bass as bass
import concourse.bacc as bacc
import concourse.tile as tile
from concourse import mybir
from contextlib import ExitStack

B, D, LC, LQ = 32, 128, 2048, 256
NCORES = 8
BPC = B // NCORES  # batches per core

F32 = mybir.dt.float32
F32R = mybir.dt.float32r
AF = mybir.ActivationFunctionType
ALU = mybir.AluOpType

IO_BUFS = 4       # all batches' inputs prefetched
OUT_BUFS = 2
BIG_BUFS = 3
SMALL_BUFS = 2
WORK_BUFS = 1


def build_nc(reps=1, hw_loop=False, unroll=1):
    nc = bacc.Bacc("TRN2", target_bir_lowering=False)
    # C/Q declared float32r (same 32-bit layout as float32): the DMA loads
    # then land fp32r tiles directly, so PE can consume them with no
    # rounding copies; elementwise consumers use a bitcast-to-f32 view.
    C_in = nc.declare_dram_parameter("C", [BPC, D, LC], F32R, isOutput=False)
    Q_in = nc.declare_dram_parameter("Q", [BPC, D, LQ], F32R, isOutput=False)
    # packed [w4mlu | w4C | w4Q] so the prologue is one tiny DMA
    w4_in = nc.declare_dram_parameter("w4", [D, 3], F32, isOutput=False)
    out_ext = nc.declare_dram_parameter("out", [BPC, 4 * D, LC], F32, isOutput=True)

    with ExitStack() as ctx:
        tc = ctx.enter_context(tile.TileContext(nc))
        singles = ctx.enter_context(tc.tile_pool(name="singles", bufs=1))
        io = ctx.enter_context(tc.tile_pool(name="io", bufs=IO_BUFS))
        outp = ctx.enter_context(tc.tile_pool(name="outp", bufs=OUT_BUFS))
        work = ctx.enter_context(tc.tile_pool(name="work", bufs=WORK_BUFS))
        psum = ctx.enter_context(tc.tile_pool(name="psum", bufs=1, space="PSUM"))

        ident = singles.tile([128, 128], F32)
        nc.gpsimd.memset(ident, 0.0)
        nc.gpsimd.affine_select(
            out=ident, in_=ident, compare_op=ALU.not_equal, fill=1.0,
            base=0, pattern=[[-1, 128]], channel_multiplier=1)
        identr = singles.tile([128, 128], F32R)
        nc.vector.tensor_copy(out=identr, in_=ident)
        ones_f = singles.tile([128, 128], F32)
        nc.vector.memset(ones_f, 1.0)
        ones = singles.tile([128, 128], F32R)
        nc.vector.tensor_copy(out=ones, in_=ones_f)
        w4_sb = singles.tile([128, 3], F32)
        nc.sync.dma_start(out=w4_sb, in_=w4_in[:])
        w4mlu_sb = w4_sb[:, 0:1]
        w4C_sb = w4_sb[:, 1:2]
        w4Q_sb = w4_sb[:, 2:3]

        from contextlib import nullcontext
        assert reps % unroll == 0
        loop_cm = (tc.For_i(0, reps // unroll, 1,
                            hint_engines=(mybir.EngineType.PE,
                                          mybir.EngineType.DVE,
                                          mybir.EngineType.Activation,
                                          mybir.EngineType.SP,
                                          mybir.EngineType.Pool))
                   if hw_loop else nullcontext(0))
        with loop_cm:
         for rep in range(unroll if hw_loop else reps):
          # ---- prologue: all input loads, then DRAM->DRAM passthroughs ----
          Qsbs, Csbs = [], []
          for b in range(BPC):
              q = io.tile([128, LQ], F32R, tag="Qsb")
              nc.sync.dma_start(out=q, in_=Q_in[b])
              Qsbs.append(q)
              c = io.tile([128, LC], F32R, tag="Csb")
              if b == 0:
                  # quarter-split the critical first C load so each E2
                  # group can start as soon as its c-tiles land
                  for qtr in range(4):
                      nc.sync.dma_start(
                          out=c[:, 512 * qtr:512 * (qtr + 1)],
                          in_=C_in[b][:, 512 * qtr:512 * (qtr + 1)])
              else:
                  nc.sync.dma_start(out=c, in_=C_in[b])
              Csbs.append(c)
          # pass plane 0 fills the load->store transition; planes 1-3 are
          # issued at the batch seams (end of each batch's store section)
          # where the store stream would otherwise idle awaiting compute
          nc.sync.dma_start(out=out_ext[0, 0:128, :],
                            in_=Csbs[0][:].bitcast(F32))

          def make_ct(b):
              # Ct = C.T via 16 PE transposes; psum drained by DVE + Act
              # (GPSIMD cannot read PSUM on hardware).
              Cr = Csbs[b]
              Ct = work.tile([128, LC], F32R, tag="Ct", bufs=2)
              for g in range(2):
                  ps_ct = psum.tile([128, 1024], F32, tag="big", bufs=BIG_BUFS)
                  for j in range(8):
                      cj = g * 8 + j
                      nc.tensor.transpose(
                          ps_ct[:, 128 * j:128 * (j + 1)].bitcast(F32R),
                          Cr[:, 128 * cj:128 * (cj + 1)], identr)
                  if g == 0:
                      nc.vector.tensor_copy(
                          out=Ct[:, 1024 * g:1024 * (g + 1)], in_=ps_ct)
                  else:
                      nc.scalar.copy(
                          out=Ct[:, 1024 * g:1024 * (g + 1)], in_=ps_ct)
              return Ct

          def make_qmlu(b, engine):
              # E1 logits = (Q*w4mlu).T @ C + sub1_q: only a small [128,Lq]
              # scaled Q is needed (no [128,Lc] Caug at all)
              Qmlu = work.tile([128, LQ], F32R, tag="Qmlu", bufs=2)
              if engine == "act":
                  nc.scalar.activation(
                      out=Qmlu, in_=Qsbs[b][:].bitcast(F32),
                      func=AF.Identity, scale=w4mlu_sb)
              else:
                  nc.vector.tensor_scalar(
                      out=Qmlu, in0=Qsbs[b][:].bitcast(F32),
                      scalar1=w4mlu_sb, scalar2=None, op0=ALU.mult)
              return Qmlu

          Qmlu_next = make_qmlu(0, "dve")
          Ct_next = None  # batch 0 computes its own Ct after E1

          for b in range(BPC):
              Cr, Qr = Csbs[b], Qsbs[b]
              Qmlu, Ct = Qmlu_next, Ct_next

              # Qaug on DVE (small, unblocks E2)
              Qaug = work.tile([128, LQ], F32R, tag="Qaug")
              nc.vector.tensor_scalar(
                  out=Qaug, in0=Qr[:].bitcast(F32), scalar1=w4mlu_sb,
                  scalar2=w4C_sb, op0=ALU.mult, op1=ALU.add)

              # Qt + sub1 = Q.T @ w4Q (per-q bias of the E1 exps) share
              # one psum tile at batch start
              Qt = work.tile([128, LQ], F32R, tag="Qt")
              sub1_sb = work.tile([128, 2], F32, tag="sub1")
              ps_qt = psum.tile([128, 512], F32, tag="small",
                                bufs=SMALL_BUFS)
              for j in range(2):
                  nc.tensor.transpose(
                      ps_qt[:, 128 * j:128 * (j + 1)].bitcast(F32R),
                      Qr[:, 128 * j:128 * (j + 1)], identr)
                  nc.tensor.matmul(
                      ps_qt[:, 256 + j:257 + j],
                      Qr[:, 128 * j:128 * (j + 1)].bitcast(F32),
                      w4_sb[:, 2:3], start=True, stop=True)
              nc.vector.tensor_copy(out=Qt, in_=ps_qt[:, 0:256])
              nc.vector.tensor_copy(out=sub1_sb, in_=ps_qt[:, 256:258])

              # ---- E2[c,q] = exp(C.T @ Qaug), one tile per 4-c-tile group ----
              E2t = []
              for g in range(4):
                  ps = psum.tile([128, 1024], F32, tag="big", bufs=BIG_BUFS)
                  for j in range(4):
                      ctile = g * 4 + j
                      nc.tensor.matmul(
                          ps[:, 256 * j:256 * (j + 1)],
                          Cr[:, 128 * ctile:128 * (ctile + 1)], Qaug,
                          start=True, stop=True)
                  e = work.tile([128, 4 * LQ], F32R, tag=f"E2g{g}")
                  nc.scalar.activation(out=e, in_=ps, func=AF.Exp)
                  E2t.append(e)

              # ---- E1^T[q,c] = exp((Q*w4mlu).T @ C + sub1), (qt, c-half) --
              E1t = {}
              for g in range(2):
                  for qt in range(2):
                      ps = psum.tile([128, 1024], F32, tag="big", bufs=BIG_BUFS)
                      for cc in range(2):
                          c0 = 1024 * g + 512 * cc
                          nc.tensor.matmul(
                              ps[:, 512 * cc:512 * (cc + 1)],
                              Qmlu[:, 128 * qt:128 * (qt + 1)],
                              Cr[:, c0:c0 + 512],
                              start=True, stop=True)
                      e = work.tile([128, 1024], F32R, tag=f"E1_{qt}{g}")
                      nc.scalar.activation(out=e, in_=ps, func=AF.Exp,
                                           bias=sub1_sb[:, qt:qt + 1])
                      E1t[(qt, g)] = e

              if Ct is None:
                  Ct = make_ct(b)

              outs = outp.tile([128, 3, LC], F32, tag="outs")
              rbi = work.tile([128, LC], F32, tag="rbi")
              last = b == BPC - 1

              def do_r(g):
                  # replicated row-sums of E1 over q -> rbi = 1/r (c-half g)
                  ps = psum.tile([128, 1024], F32, tag="big", bufs=BIG_BUFS)
                  for cc in range(2):
                      for qt in range(2):
                          nc.tensor.matmul(
                              ps[:, 512 * cc:512 * (cc + 1)],
                              ones, E1t[(qt, g)][:, 512 * cc:512 * (cc + 1)],
                              start=(qt == 0), stop=(qt == 1))
                  nc.vector.reciprocal_approx_fast(
                      out=rbi[:, 1024 * g:1024 * (g + 1)], in_=ps)

              def do_ma(g):
                  # MA half -> out1 (DVE), out2 (gpsimd), store the half
                  sl = slice(1024 * g, 1024 * (g + 1))
                  ps = psum.tile([128, 1024], F32, tag="big", bufs=BIG_BUFS)
                  for cc in range(2):
                      for qt in range(2):
                          nc.tensor.matmul(
                              ps[:, 512 * cc:512 * (cc + 1)],
                              Qt[:, 128 * qt:128 * (qt + 1)],
                              E1t[(qt, g)][:, 512 * cc:512 * (cc + 1)],
                              start=(qt == 0), stop=(qt == 1))
                  nc.vector.tensor_mul(out=outs[:, 0, sl], in0=ps, in1=rbi[:, sl])
                  nc.sync.dma_start(
                      out=out_ext[b, 128:256, sl], in_=outs[:, 0, sl])
                  nc.gpsimd.tensor_mul(
                      out=outs[:, 1, sl], in0=outs[:, 0, sl],
                      in1=Cr[:, sl].bitcast(F32))
                  nc.sync.dma_start(
                      out=out_ext[b, 256:384, sl], in_=outs[:, 1, sl])

              def do_mb(g):
                  sl = slice(1024 * g, 1024 * (g + 1))
                  ps2 = psum.tile([128, 1024], F32, tag="big", bufs=BIG_BUFS)
                  for cc in range(2):
                      for qt in range(2):
                          nc.tensor.matmul(
                              ps2[:, 512 * cc:512 * (cc + 1)],
                              T_sb[:, 128 * qt:128 * (qt + 1)],
                              E1t[(qt, g)][:, 512 * cc:512 * (cc + 1)],
                              start=(qt == 0), stop=(qt == 1))
                  MBr = work.tile([128, 1024], F32, tag="MBr", bufs=2)
                  nc.vector.tensor_mul(out=MBr, in0=ps2, in1=rbi[:, sl])
                  nc.gpsimd.tensor_mul(
                      out=outs[:, 2, sl], in0=MBr,
                      in1=Cr[:, sl].bitcast(F32))
                  if last:
                      nc.sync.dma_start(
                          out=out_ext[b, 384:512, sl], in_=outs[:, 2, sl])

              # ---- c-half pipeline: g0's r/MA (and its stores) run before
              # the full-c reductions (s/MT) so output DMA starts early ----
              do_r(0)
              do_ma(0)

              # ---- s (col-sums of E2) and MT^T = Ct.T @ E2, interleaved by
              # group so PE consumption tracks the Act exp stream ----
              sinv = work.tile([128, LQ], F32, tag="sinv")
              MTs = work.tile([128, LQ], F32R, tag="MTs")
              ps_s = psum.tile([128, 512], F32, tag="small", bufs=SMALL_BUFS)
              ps_mt = psum.tile([128, 512], F32, tag="small", bufs=SMALL_BUFS)
              for g in range(4):
                  for j in range(4):
                      blk = 4 * g + j
                      nc.tensor.matmul(
                          ps_s[:, 0:256], ones, E2t[g][:, 256 * j:256 * (j + 1)],
                          start=(blk == 0), stop=(blk == 15))
                  for j in range(4):
                      blk = 4 * g + j
                      nc.tensor.matmul(
                          ps_mt[:, 0:256],
                          Ct[:, 128 * blk:128 * (blk + 1)],
                          E2t[g][:, 256 * j:256 * (j + 1)],
                          start=(blk == 0), stop=(blk == 15))
              do_r(1)
              do_ma(1)

              nc.vector.reciprocal_approx_fast(out=sinv, in_=ps_s[:, 0:256])
              nc.vector.tensor_mul(out=MTs, in0=ps_mt[:, 0:256], in1=sinv)

              # ---- T = transpose(MTs): [q, d] for MB's stationary operand ----
              T_sb = work.tile([128, LQ], F32R, tag="T_sb")
              ps_t = psum.tile([128, 512], F32, tag="small", bufs=SMALL_BUFS)
              for j in range(2):
                  nc.tensor.transpose(
                      ps_t[:, 128 * j:128 * (j + 1)].bitcast(F32R),
                      MTs[:, 128 * j:128 * (j + 1)], identr)
              nc.scalar.copy(out=T_sb, in_=ps_t[:, 0:256])

              do_mb(0)
              do_mb(1)

              # software-pipelined Ct/Caug for the next batch: PE fills the
              # gap while DVE drains the MB psums; Act appends Caug after
              # its exp stream so exps never queue behind it
              if b + 1 < BPC:
                  Ct_next = make_ct(b + 1)
                  Qmlu_next = make_qmlu(b + 1, "act")

              if not last:
                  nc.sync.dma_start(
                      out=out_ext[b, 384:512, :], in_=outs[:, 2, :])
                  nc.sync.dma_start(
                      out=out_ext[b + 1, 0:128, :],
                      in_=Csbs[b + 1][:].bitcast(F32))

    nc.compile()
    return nc


_NC = {}


def _get_nc(reps=1, hw_loop=False, unroll=1):
    key = (reps, hw_loop, unroll)
    if key not in _NC:
        _NC[key] = build_nc(reps, hw_loop, unroll)
    return _NC[key]


def make_in_maps(C, Q, w4C, w4Q, w4mlu):
    C = np.ascontiguousarray(np.asarray(C), dtype=np.float32)
    Q = np.ascontiguousarray(np.asarray(Q), dtype=np.float32)
    w4C = np.asarray(w4C, dtype=np.float32).reshape(D, 1)
    w4Q = np.asarray(w4Q, dtype=np.float32).reshape(D, 1)
    w4mlu = np.asarray(w4mlu, dtype=np.float32).reshape(D, 1)
    w4 = np.ascontiguousarray(np.concatenate([w4mlu, w4C, w4Q], axis=1))
    in_maps = []
    for i in range(NCORES):
        sl = slice(i * BPC, (i + 1) * BPC)
        in_maps.append({
            "C": np.ascontiguousarray(C[sl]),
            "Q": np.ascontiguousarray(Q[sl]),
            "w4": w4,
        })
    return in_maps


def run(C, Q, w4C, w4Q, w4mlu, trace=False, tmpdir=None):
    from concourse.bass_utils import run_bass_kernel_spmd
    nc = _get_nc()
    in_maps = make_in_maps(C, Q, w4C, w4Q, w4mlu)
    res = run_bass_kernel_spmd(
        nc, in_maps, list(range(NCORES)), trace=trace, tmpdir=tmpdir)
    out = np.concatenate(
        [res.results[i]["out"] for i in range(NCORES)], axis=0)
    return out, res


def kernel(C, Q, Cmask=None, Qmask=None, w4C=None, w4Q=None, w4mlu=None,
           bias=None, **_unused):
    # Cmask/Qmask are all-ones in this problem and bias cancels exactly in
    # every output (softmax shift invariance), so neither reaches the device.
    out, _ = run(C, Q, w4C, w4Q, w4mlu)
    return out

